# revision 2
# baseline (speedup 1.0000x reference)
"""Bidirectional LSTM (L=512, B=64, E=512, H=512 per dir) on 8 NeuronCores.

Strategy (SPMD, zero cross-core communication):
  - Direction x batch sharding: cores 0-3 run the FORWARD direction for
    batches [16c, 16c+16); cores 4-7 run the BACKWARD direction (inputs
    time-reversed on host) for batches [16(c-4), 16(c-4)+16). One Bass
    program for all cores; only the host-prepared inputs differ.
  - Orientation: gate dims live in the PARTITION dim (16 m-tiles of 128,
    gate order [f|i|o|g], each gate 4 h-chunks), batch=16 in the free dim.
    Per step, gates[128,256] accumulate in one PSUM tile:
      inject bias (K=16 matmul vs a static delta pattern) and the padding
      penalty -1e9*(1-mask) (K=1 matmul, f/i/o tiles only), then 64 x-part
      matmuls (Wih^T chunks @ X^T cols) and 64 h-part matmuls, all N=16.
    sigmoid(-1e9)=0 reproduces the reference's h,c masking exactly.
  - h is produced as [128 h-dims, 16 batch] bf16 -- exactly the moving
    operand layout the next step's h-matmuls need: no transposes in the
    recurrence.
  - Embedding rows are gathered by indirect DMA (8 steps worth per tile),
    PE-transposed into a persistent bf16 X^T[128, 4, L*16] in SBUF,
    pipelined ahead of the recurrence.
  - All matmul operands bf16; PSUM accumulation and the c state stay fp32.
"""

import os
import sys

sys.path.insert(0, "/opt/trn_rl_repo")

import numpy as np

L, B, E, V = 512, 64, 512, 32000
H = 512            # hidden per direction
NB = 16            # batch per core
NCORES = 8
NM = 16            # m-tiles (2048 gate dims / 128)

_BUILT = {}


def _split_sync_waits(nc, max_waits=1):
    """This container's walrus rejects >1 sync-wait per instruction
    (CoreV3GenImpl setupSyncWait). Split extras onto preceding same-engine
    NoOps. Keep the *most recently required* wait (highest wait_value
    relative to that semaphore's final count) on the instruction itself, so
    the NoOps carry long-satisfied waits and drain through the sequencer
    without stalling the critical path."""
    import concourse.mybir as mybir

    # final tick per semaphore id + whether it is only updated by DMA-ish
    # instructions (those sems are long-satisfied WAR guards, never the
    # chain-critical producer)
    total = {}
    dma_only = {}
    for fn in nc.m.functions:
        for blk in fn.blocks:
            for inst in blk.instructions:
                si = inst.sync_info
                if si is None:
                    continue
                is_dma = "DMA" in type(inst).__name__ or "Dma" in type(
                    inst).__name__
                for u in si.on_update:
                    v = total.get(u.id, 0)
                    total[u.id] = v + (u.update_value or 1)
                    dma_only[u.id] = dma_only.get(u.id, True) and is_dma

    def recency(w):
        t = total.get(w.id, 0)
        r = (w.wait_value or 0) / t if t else 0.0
        return (0 if dma_only.get(w.id, False) else 1, r)

    ctr = 0
    for fn in nc.m.functions:
        for blk in fn.blocks:
            out = []
            changed = False
            for inst in blk.instructions:
                si = inst.sync_info
                if si is not None and si.on_wait and len(si.on_wait) > max_waits:
                    waits = sorted(si.on_wait, key=recency)
                    extra, keep = waits[:-max_waits], waits[-max_waits:]
                    for i in range(0, len(extra), max_waits):
                        ctr += 1
                        nop = mybir.InstNoOp(
                            name=f"bass_waitsplit_{ctr}", ins=[], outs=[])
                        nop.engine = inst.engine
                        nop.sync_info = mybir.SyncInfo(
                            on_wait=extra[i:i + max_waits], on_update=[])
                        out.append(nop)
                    si.on_wait = keep
                    changed = True
                out.append(inst)
            if changed:
                blk.instructions[:] = out


# gate order [g, i, f, o] referencing reference row blocks i=0:512,
# f=512:1024, g=1024:1536, o=1536:2048. g,i first so sigmoid over [g|i]
# can start after only half the h-part matmuls.
_GATE_BASES = (1024, 0, 512, 1536)


def _gate_perm():
    return np.concatenate([np.arange(b, b + 512) for b in _GATE_BASES])


def _build(nsteps=L):
    key = (nsteps, NB)
    if key in _BUILT:
        return _BUILT[key]
    import concourse.bass as bass
    import concourse.mybir as mybir
    import concourse.tile as tile
    from concourse.masks import make_identity

    f32 = mybir.dt.float32
    bf16 = mybir.dt.bfloat16
    ntiles = nsteps * NB // 128   # gather row-tiles (8 steps each)
    npc = (nsteps + 31) // 32     # penalty col-chunks (32 steps each)

    nc = bass.Bass()
    emb = nc.dram_tensor("emb", [V, E], f32, kind="ExternalInput")
    toks = nc.dram_tensor("toks", [128, ntiles], mybir.dt.int32,
                          kind="ExternalInput")
    wihT_d = nc.dram_tensor("wihT", [4, 128, 2048], bf16, kind="ExternalInput")
    whhT_d = nc.dram_tensor("whhT", [4, 128, 2048], bf16, kind="ExternalInput")
    biasL_d = nc.dram_tensor("biasL", [16, 128], bf16, kind="ExternalInput")
    delta_d = nc.dram_tensor("delta", [16, 256], bf16, kind="ExternalInput")
    sel_d = nc.dram_tensor("sel", [32, 32 * 128], bf16, kind="ExternalInput")
    pen_d = nc.dram_tensor("pen", [32, npc * 192], bf16, kind="ExternalInput")
    out_d = nc.dram_tensor("out", [nsteps, 128, 64], bf16,
                           kind="ExternalOutput")

    PF = 2  # prefetch distance in row-tiles (gather 2*PF ahead, transpose PF)

    with tile.TileContext(nc) as tc:
        with (
            tc.tile_pool(name="persist", bufs=1) as pp,
            tc.tile_pool(name="gat", bufs=2 * PF + 1) as gat,
            tc.tile_pool(name="step", bufs=3) as sp,
            tc.tile_pool(name="state", bufs=3) as st,
            tc.tile_pool(name="psA", bufs=2, space="PSUM") as psA,
            tc.tile_pool(name="psB", bufs=2, space="PSUM") as psB,
            tc.tile_pool(name="psC", bufs=2, space="PSUM") as psC,
            tc.tile_pool(name="pstr", bufs=2, space="PSUM") as pstr,
        ):
            # ---- persistent SBUF ----
            wihT = pp.tile([128, 4, 2048], bf16)
            whhT = pp.tile([128, 4, 2048], bf16)
            for k in range(4):
                nc.sync.dma_start(wihT[:, k, :], wihT_d[k])
                nc.sync.dma_start(whhT[:, k, :], whhT_d[k])
            biasL = pp.tile([16, 128], bf16)
            nc.sync.dma_start(biasL[:], biasL_d[:])
            delta = pp.tile([16, 256], bf16)
            nc.sync.dma_start(delta[:], delta_d[:])
            pen = pp.tile([32, npc * 192], bf16)
            nc.sync.dma_start(pen[:], pen_d[:])
            sel = pp.tile([32, 32 * 128], bf16)
            nc.sync.dma_start(sel[:], sel_d[:])
            toks_t = pp.tile([128, ntiles], mybir.dt.int32)
            nc.sync.dma_start(toks_t[:], toks[:])
            ident = pp.tile([128, 128], f32)
            make_identity(nc, ident[:])
            xT = pp.tile([128, 4, nsteps * NB], bf16)

            def gather(r):
                xt = gat.tile([128, E], f32, tag="xt")
                nc.gpsimd.indirect_dma_start(
                    out=xt[:], out_offset=None, in_=emb[:],
                    in_offset=bass.IndirectOffsetOnAxis(
                        ap=toks_t[:, r:r + 1], axis=0))
                return xt

            def transpose_tile(r, xt):
                for k in range(4):
                    trp = pstr.tile([128, 128], f32, space="PSUM")
                    nc.tensor.transpose(
                        out=trp[:], in_=xt[:, 128 * k:128 * (k + 1)],
                        identity=ident[:])
                    dst = xT[:, k, 128 * r:128 * (r + 1)]
                    if k % 2 == 0:
                        nc.vector.tensor_copy(dst, trp[:])
                    else:
                        nc.scalar.copy(dst, trp[:])

            xts = {r: gather(r) for r in range(min(2 * PF, ntiles))}
            for r in range(min(PF, ntiles)):
                transpose_tile(r, xts.pop(r))

            h_prev = None
            c_prev = None
            for t in range(nsteps):
                if t % 8 == 0:
                    r = t // 8
                    if r + 2 * PF < ntiles:
                        xts[r + 2 * PF] = gather(r + 2 * PF)
                    if r + PF < ntiles:
                        transpose_tile(r + PF, xts.pop(r + PF))

                # Gate groups in separate PSUM banks so each sigmoid is only
                # bank-serialized against its own gates' matmul writes:
                # bank A = [g|i] (m 0..7), bank B = f (m 8..11), C = o (12..15)
                gA = psA.tile([128, 512], f32, space="PSUM")
                gB = psB.tile([128, 512], f32, space="PSUM")
                gC = psC.tile([128, 512], f32, space="PSUM")
                banks = [gA] * 8 + [gB] * 4 + [gC] * 4
                boff = [16 * m for m in range(8)] + [16 * m for m in range(4)] * 2

                def gslice(m):
                    return banks[m][:, boff[m]:boff[m] + 16]

                cc = 192 * (t // 32)
                selt = sel[:, 128 * (t % 32):128 * (t % 32) + 128]
                # bias inject: out[p, 16m+b] += biasL[m, p]
                nc.tensor.matmul(gA[:, 0:128], biasL[:, :], delta[:, 0:128],
                                 start=True, stop=False, skip_group_check=True)
                nc.tensor.matmul(gB[:, 0:64], biasL[:, :], delta[:, 128:192],
                                 start=True, stop=False, skip_group_check=True)
                nc.tensor.matmul(gC[:, 0:64], biasL[:, :], delta[:, 192:256],
                                 start=True, stop=False, skip_group_check=True)
                # penalty inject on i/f/o tiles (m=4..15): += P[t, b]
                # lhsT selects pen row t%32 (sel row t%32 is all-ones)
                nc.tensor.matmul(gA[:, 64:128], selt, pen[:, cc:cc + 64],
                                 start=False, stop=False, skip_group_check=True)
                nc.tensor.matmul(gB[:, 0:64], selt, pen[:, cc + 64:cc + 128],
                                 start=False, stop=False, skip_group_check=True)
                nc.tensor.matmul(gC[:, 0:64], selt, pen[:, cc + 128:cc + 192],
                                 start=False, stop=False, skip_group_check=True)
                # x part: gates += Wih^T[k,m] @ xT[k][:, t]
                for k in range(4):
                    rhs = xT[:, k, NB * t:NB * (t + 1)]
                    for m in range(NM):
                        nc.tensor.matmul(
                            gslice(m),
                            wihT[:, k, 128 * m:128 * (m + 1)], rhs,
                            start=False,
                            stop=(h_prev is None and k == 3),
                            skip_group_check=True)
                # h part: gates += Whh^T[k,m] @ h_prev[:, k].
                # f tiles (m 8..11) first: sig_f -> t2 is the longest
                # downstream path; then g,i (t1), o last.
                if h_prev is not None:
                    for m in (8, 9, 10, 11, 0, 1, 2, 3, 4, 5, 6, 7,
                              12, 13, 14, 15):
                        for k in range(4):
                            nc.tensor.matmul(
                                gslice(m),
                                whhT[:, k, 128 * m:128 * (m + 1)],
                                h_prev[:, 16 * k:16 * (k + 1)],
                                start=False, stop=(k == 3),
                                skip_group_check=True)

                # Gate cols: g 0:64, i 64:128, f 128:192, o 192:256.
                # g rows were pre-scaled x2 on the host, so
                # tanh(g) = 2*sig(2g) - 1; we track c' = c/2:
                #   c' = sig_f*c'_prev + (sig_2g-0.5)*sig_i, h = sig_o*tanh(2c')
                # Two sigmoid halves: [g|i] can fire after half the h-matmuls.
                sigF = sp.tile([128, 64], f32, tag="sigF")
                nc.scalar.activation(sigF[:], gB[:, 0:64],
                                     mybir.ActivationFunctionType.Sigmoid)
                sigA = sp.tile([128, 128], f32, tag="sigA")
                nc.scalar.activation(sigA[:], gA[:, 0:128],
                                     mybir.ActivationFunctionType.Sigmoid)
                sigO = sp.tile([128, 64], f32, tag="sigO")
                nc.scalar.activation(sigO[:], gC[:, 0:64],
                                     mybir.ActivationFunctionType.Sigmoid)

                c_new = st.tile([128, 64], f32, tag="c")
                if c_prev is None:
                    nc.vector.scalar_tensor_tensor(
                        c_new[:], sigA[:, 0:64], 0.5, sigA[:, 64:128],
                        mybir.AluOpType.subtract, mybir.AluOpType.mult)
                else:
                    t2 = sp.tile([128, 64], f32, tag="t2")
                    nc.vector.tensor_mul(t2[:], sigF[:], c_prev[:])
                    t1 = sp.tile([128, 64], f32, tag="t1")
                    nc.vector.scalar_tensor_tensor(
                        t1[:], sigA[:, 0:64], 0.5, sigA[:, 64:128],
                        mybir.AluOpType.subtract, mybir.AluOpType.mult)
                    nc.vector.tensor_add(c_new[:], t1[:], t2[:])
                tc_ = sp.tile([128, 64], f32, tag="tc")
                nc.scalar.activation(tc_[:], c_new[:],
                                     mybir.ActivationFunctionType.Tanh,
                                     scale=2.0)
                h = st.tile([128, 64], bf16, tag="h")
                nc.vector.tensor_mul(h[:], sigO[:], tc_[:])

                nc.sync.dma_start(out_d[t], h[:])

                h_prev = h
                c_prev = c_new

    _BUILT[key] = nc
    return nc


def _ensure_split(nc):
    if not getattr(nc, "_waitsplit_done", False):
        _split_sync_waits(nc)
        nc._waitsplit_done = True


def _prep_core_inputs(c, tokens, mask, emb_table, wihT_f, whhT_f, biasL_f,
                      wihT_b, whhT_b, biasL_b, delta, nsteps):
    import ml_dtypes

    bf16 = ml_dtypes.bfloat16
    backward = c >= 4
    s = slice(NB * (c % 4), NB * (c % 4) + NB)
    tok = np.asarray(tokens)[:nsteps, s]
    msk = np.asarray(mask)[:nsteps, s]
    if backward:
        tok = tok[::-1]
        msk = msk[::-1]
    ntiles = nsteps * NB // 128
    toks_c = np.clip(tok, 0, V - 1).astype(np.int32).reshape(ntiles, 128).T
    # pen[j, 192*cc + 16*i + b] = -1e9 * (1 - mask[32*cc + j, b]), i<12
    npc = (nsteps + 31) // 32
    P = np.zeros((npc * 32, NB), np.float32)
    P[:nsteps] = -1e9 * (1.0 - msk.astype(np.float32))
    pen = np.tile(P.reshape(npc, 32, 1, NB), (1, 1, 12, 1))
    pen = pen.transpose(1, 0, 2, 3).reshape(32, npc * 192)
    # sel[j, 128*jj + p] = (j == jj)
    sel = np.repeat(np.eye(32, dtype=np.float32), 128, axis=1)
    return {
        "emb": np.ascontiguousarray(emb_table),
        "toks": np.ascontiguousarray(toks_c),
        "wihT": wihT_b if backward else wihT_f,
        "whhT": whhT_b if backward else whhT_f,
        "biasL": biasL_b if backward else biasL_f,
        "delta": delta,
        "sel": np.ascontiguousarray(sel.astype(bf16)),
        "pen": np.ascontiguousarray(pen.astype(bf16)),
    }


def kernel(tokens, mask, emb_table, W_ih_f, W_hh_f, b_ih_f, b_hh_f,
           W_ih_b, W_hh_b, b_ih_b, b_hh_b, _nsteps=L, _trace=False):
    import ml_dtypes
    from concourse.bass_utils import run_bass_kernel_spmd

    bf16 = ml_dtypes.bfloat16
    tokens = np.asarray(tokens)
    mask = np.asarray(mask, dtype=np.float32)
    emb_table = np.asarray(emb_table, dtype=np.float32)

    perm = _gate_perm()

    # g-gate rows (first 512 after perm) pre-scaled x2: tanh(g)=2*sig(2g)-1
    gscale = np.ones((2048, 1), np.float32)
    gscale[0:512] = 2.0

    def wprep(W):
        Wp = np.asarray(W, np.float32)[perm] * gscale
        return np.ascontiguousarray(Wp.T.reshape(4, 128, 2048).astype(bf16))

    def bprep(bi, bh):
        b = (np.asarray(bi, np.float32) + np.asarray(bh, np.float32))[perm]
        b = b * gscale[:, 0]
        return np.ascontiguousarray(b.reshape(16, 128).astype(bf16))

    wihT_f, whhT_f = wprep(W_ih_f), wprep(W_hh_f)
    wihT_b, whhT_b = wprep(W_ih_b), wprep(W_hh_b)
    biasL_f = bprep(b_ih_f, b_hh_f)
    biasL_b = bprep(b_ih_b, b_hh_b)
    delta = np.ascontiguousarray(
        np.kron(np.eye(16, dtype=np.float32),
                np.ones((1, 16), np.float32)).astype(bf16))

    nsteps = _nsteps
    nc = _build(nsteps)
    _ensure_split(nc)
    in_maps = [
        _prep_core_inputs(c, tokens, mask, emb_table, wihT_f, whhT_f, biasL_f,
                          wihT_b, whhT_b, biasL_b, delta, nsteps)
        for c in range(NCORES)
    ]
    res = run_bass_kernel_spmd(nc, in_maps, core_ids=list(range(NCORES)),
                               trace=_trace)
    out = np.empty((nsteps, B, 2 * H), np.float32)
    for c in range(NCORES):
        o = np.asarray(res.results[c]["out"]).astype(np.float32)
        # o[t, p, 16j+b] -> h[t, b, 128j+p]
        o = o.reshape(nsteps, 128, 4, NB).transpose(0, 3, 2, 1)
        o = np.ascontiguousarray(o).reshape(nsteps, NB, 512)
        s = slice(NB * (c % 4), NB * (c % 4) + NB)
        if c >= 4:
            out[:, s, 512:1024] = o[::-1]
        else:
            out[:, s, 0:512] = o
    kernel._last_results = res
    return out


# revision 4
# speedup vs baseline: 1.0262x; 1.0262x over previous
"""Bidirectional LSTM (L=512, B=64, E=512, H=512 per dir) on 8 NeuronCores.

Strategy (SPMD, zero cross-core communication):
  - Direction x batch sharding: cores 0-3 run the FORWARD direction for
    batches [16c, 16c+16); cores 4-7 run the BACKWARD direction (inputs
    time-reversed on host) for batches [16(c-4), 16(c-4)+16). One Bass
    program for all cores; only the host-prepared inputs differ.
  - Orientation: gate dims live in the PARTITION dim (16 m-tiles of 128,
    gate order [g|i|f|o], each gate 4 h-chunks), batch=16 in the free dim.
    Per step, gates accumulate in three PSUM banks (A=[g|i], B=f, C=o, so
    each sigmoid is only bank-serialized against its own gates' writes):
      inject bias (K=16 matmul vs a static delta pattern) and the padding
      penalty -1e9*(1-mask) (K=32 row-select matmul, i/f/o tiles only),
      then 64 x-part matmuls (Wih^T chunks @ X^T cols) and 64 h-part
      matmuls (f tiles first -- sig_f gates the longest downstream path),
      all N=16. sigmoid(-1e9)=0 reproduces the reference's h,c masking.
  - g rows are pre-scaled x2 on the host so all four gates need only
    sigmoid (tanh(g) = 2*sig(2g)-1), tracking c' = c/2:
      c' = sig_f*c' + (sig_2g-0.5)*sig_i;  h = sig_o*tanh(2c').
  - h is produced as [128 h-dims, 16 batch] bf16 -- exactly the moving
    operand layout the next step's h-matmuls need: no transposes in the
    recurrence.
  - Embedding rows are gathered by indirect DMA (8 steps worth per tile),
    PE-transposed into a persistent bf16 X^T[128, 4, L*16] in SBUF,
    pipelined ahead of the recurrence.
  - All matmul operands bf16; PSUM accumulation and the c state stay fp32.
"""

import os
import sys

sys.path.insert(0, "/opt/trn_rl_repo")

import numpy as np

L, B, E, V = 512, 64, 512, 32000
H = 512            # hidden per direction
NB = 16            # batch per core
NCORES = 8
NM = 16            # m-tiles (2048 gate dims / 128)

_BUILT = {}


def _split_sync_waits(nc, max_waits=1):
    """This container's walrus rejects >1 sync-wait per instruction
    (CoreV3GenImpl setupSyncWait). Split extras onto preceding same-engine
    NoOps. Keep the *most recently required* wait (highest wait_value
    relative to that semaphore's final count) on the instruction itself, so
    the NoOps carry long-satisfied waits and drain through the sequencer
    without stalling the critical path."""
    import concourse.mybir as mybir

    # final tick per semaphore id + whether it is only updated by DMA-ish
    # instructions (those sems are long-satisfied WAR guards, never the
    # chain-critical producer)
    total = {}
    dma_only = {}
    for fn in nc.m.functions:
        for blk in fn.blocks:
            for inst in blk.instructions:
                si = inst.sync_info
                if si is None:
                    continue
                is_dma = "DMA" in type(inst).__name__ or "Dma" in type(
                    inst).__name__
                for u in si.on_update:
                    v = total.get(u.id, 0)
                    total[u.id] = v + (u.update_value or 1)
                    dma_only[u.id] = dma_only.get(u.id, True) and is_dma

    def recency(w):
        t = total.get(w.id, 0)
        r = (w.wait_value or 0) / t if t else 0.0
        return (0 if dma_only.get(w.id, False) else 1, r)

    ctr = 0
    for fn in nc.m.functions:
        for blk in fn.blocks:
            out = []
            changed = False
            for inst in blk.instructions:
                si = inst.sync_info
                if si is not None and si.on_wait and len(si.on_wait) > max_waits:
                    waits = sorted(si.on_wait, key=recency)
                    extra, keep = waits[:-max_waits], waits[-max_waits:]
                    for i in range(0, len(extra), max_waits):
                        ctr += 1
                        nop = mybir.InstNoOp(
                            name=f"bass_waitsplit_{ctr}", ins=[], outs=[])
                        nop.engine = inst.engine
                        nop.sync_info = mybir.SyncInfo(
                            on_wait=extra[i:i + max_waits], on_update=[])
                        out.append(nop)
                    si.on_wait = keep
                    changed = True
                out.append(inst)
            if changed:
                blk.instructions[:] = out


# gate order [g, i, f, o] referencing reference row blocks i=0:512,
# f=512:1024, g=1024:1536, o=1536:2048. g,i first so sigmoid over [g|i]
# can start after only half the h-part matmuls.
_GATE_BASES = (1024, 0, 512, 1536)


def _gate_perm():
    return np.concatenate([np.arange(b, b + 512) for b in _GATE_BASES])


def _build(nsteps=L):
    key = (nsteps, NB)
    if key in _BUILT:
        return _BUILT[key]
    import concourse.bass as bass
    import concourse.mybir as mybir
    import concourse.tile as tile
    from concourse.masks import make_identity

    f32 = mybir.dt.float32
    bf16 = mybir.dt.bfloat16
    ntiles = nsteps * NB // 128   # gather row-tiles (8 steps each)
    npc = (nsteps + 31) // 32     # penalty col-chunks (32 steps each)

    nc = bass.Bass()
    emb = nc.dram_tensor("emb", [V, E], f32, kind="ExternalInput")
    toks = nc.dram_tensor("toks", [128, ntiles], mybir.dt.int32,
                          kind="ExternalInput")
    wihT_d = nc.dram_tensor("wihT", [4, 128, 2048], bf16, kind="ExternalInput")
    whhT_d = nc.dram_tensor("whhT", [4, 128, 2048], bf16, kind="ExternalInput")
    biasL_d = nc.dram_tensor("biasL", [16, 128], bf16, kind="ExternalInput")
    delta_d = nc.dram_tensor("delta", [16, 256], bf16, kind="ExternalInput")
    sel_d = nc.dram_tensor("sel", [32, 32 * 128], bf16, kind="ExternalInput")
    pen_d = nc.dram_tensor("pen", [32, npc * 192], bf16, kind="ExternalInput")
    out_d = nc.dram_tensor("out", [nsteps, 128, 64], bf16,
                           kind="ExternalOutput")

    PF = 2  # prefetch distance in row-tiles (gather 2*PF ahead, transpose PF)

    with tile.TileContext(nc) as tc:
        with (
            tc.tile_pool(name="persist", bufs=1) as pp,
            tc.tile_pool(name="gat", bufs=2 * PF + 1) as gat,
            tc.tile_pool(name="step", bufs=3) as sp,
            tc.tile_pool(name="state", bufs=3) as st,
            tc.tile_pool(name="psA", bufs=2, space="PSUM") as psA,
            tc.tile_pool(name="psB", bufs=2, space="PSUM") as psB,
            tc.tile_pool(name="psC", bufs=2, space="PSUM") as psC,
            tc.tile_pool(name="pstr", bufs=2, space="PSUM") as pstr,
        ):
            # ---- persistent SBUF ----
            wihT = pp.tile([128, 4, 2048], bf16)
            whhT = pp.tile([128, 4, 2048], bf16)
            for k in range(4):
                nc.sync.dma_start(wihT[:, k, :], wihT_d[k])
                nc.sync.dma_start(whhT[:, k, :], whhT_d[k])
            biasL = pp.tile([16, 128], bf16)
            nc.sync.dma_start(biasL[:], biasL_d[:])
            delta = pp.tile([16, 256], bf16)
            nc.sync.dma_start(delta[:], delta_d[:])
            pen = pp.tile([32, npc * 192], bf16)
            nc.sync.dma_start(pen[:], pen_d[:])
            sel = pp.tile([32, 32 * 128], bf16)
            nc.sync.dma_start(sel[:], sel_d[:])
            toks_t = pp.tile([128, ntiles], mybir.dt.int32)
            nc.sync.dma_start(toks_t[:], toks[:])
            ident = pp.tile([128, 128], f32)
            make_identity(nc, ident[:])
            xT = pp.tile([128, 4, nsteps * NB], bf16)

            def gather(r):
                xt = gat.tile([128, E], f32, tag="xt")
                nc.gpsimd.indirect_dma_start(
                    out=xt[:], out_offset=None, in_=emb[:],
                    in_offset=bass.IndirectOffsetOnAxis(
                        ap=toks_t[:, r:r + 1], axis=0))
                return xt

            def transpose_tile(r, xt):
                # copies stay off ACT: ACT is the chain-critical engine
                for k in range(4):
                    trp = pstr.tile([128, 128], f32, space="PSUM")
                    nc.tensor.transpose(
                        out=trp[:], in_=xt[:, 128 * k:128 * (k + 1)],
                        identity=ident[:])
                    nc.vector.tensor_copy(xT[:, k, 128 * r:128 * (r + 1)],
                                          trp[:])

            xts = {r: gather(r) for r in range(min(2 * PF, ntiles))}
            for r in range(min(PF, ntiles)):
                transpose_tile(r, xts.pop(r))

            h_prev = None
            c_prev = None
            for t in range(nsteps):
                if t % 8 == 0:
                    r = t // 8
                    if r + 2 * PF < ntiles:
                        xts[r + 2 * PF] = gather(r + 2 * PF)
                    if r + PF < ntiles:
                        transpose_tile(r + PF, xts.pop(r + PF))

                # Gate groups in separate PSUM banks so each sigmoid is only
                # bank-serialized against its own gates' matmul writes:
                # bank A = [g|i] (m 0..7), bank B = f (m 8..11), C = o (12..15)
                gA = psA.tile([128, 512], f32, space="PSUM")
                gB = psB.tile([128, 512], f32, space="PSUM")
                gC = psC.tile([128, 512], f32, space="PSUM")
                banks = [gA] * 8 + [gB] * 4 + [gC] * 4
                boff = [16 * m for m in range(8)] + [16 * m for m in range(4)] * 2

                def gslice(m):
                    return banks[m][:, boff[m]:boff[m] + 16]

                cc = 192 * (t // 32)
                selt = sel[:, 128 * (t % 32):128 * (t % 32) + 128]
                # bias inject: out[p, 16m+b] += biasL[m, p]
                nc.tensor.matmul(gA[:, 0:128], biasL[:, :], delta[:, 0:128],
                                 start=True, stop=False, skip_group_check=True)
                nc.tensor.matmul(gB[:, 0:64], biasL[:, :], delta[:, 128:192],
                                 start=True, stop=False, skip_group_check=True)
                nc.tensor.matmul(gC[:, 0:64], biasL[:, :], delta[:, 192:256],
                                 start=True, stop=False, skip_group_check=True)
                # penalty inject on i/f/o tiles (m=4..15): += P[t, b]
                # lhsT selects pen row t%32 (sel row t%32 is all-ones)
                nc.tensor.matmul(gA[:, 64:128], selt, pen[:, cc:cc + 64],
                                 start=False, stop=False, skip_group_check=True)
                nc.tensor.matmul(gB[:, 0:64], selt, pen[:, cc + 64:cc + 128],
                                 start=False, stop=False, skip_group_check=True)
                nc.tensor.matmul(gC[:, 0:64], selt, pen[:, cc + 128:cc + 192],
                                 start=False, stop=False, skip_group_check=True)
                # x part: gates += Wih^T[k,m] @ xT[k][:, t]
                for k in range(4):
                    rhs = xT[:, k, NB * t:NB * (t + 1)]
                    for m in range(NM):
                        nc.tensor.matmul(
                            gslice(m),
                            wihT[:, k, 128 * m:128 * (m + 1)], rhs,
                            start=False,
                            stop=(h_prev is None and k == 3),
                            skip_group_check=True)
                # h part: gates += Whh^T[k,m] @ h_prev[:, k].
                # f tiles (m 8..11) first: sig_f -> t2 is the longest
                # downstream path; then g,i (t1), o last.
                if h_prev is not None:
                    for m in (8, 9, 10, 11, 0, 1, 2, 3, 4, 5, 6, 7,
                              12, 13, 14, 15):
                        for k in range(4):
                            nc.tensor.matmul(
                                gslice(m),
                                whhT[:, k, 128 * m:128 * (m + 1)],
                                h_prev[:, 16 * k:16 * (k + 1)],
                                start=False, stop=(k == 3),
                                skip_group_check=True)

                # Gate cols: g 0:64, i 64:128, f 128:192, o 192:256.
                # g rows were pre-scaled x2 on the host, so
                # tanh(g) = 2*sig(2g) - 1; we track c' = c/2:
                #   c' = sig_f*c'_prev + (sig_2g-0.5)*sig_i, h = sig_o*tanh(2c')
                # Two sigmoid halves: [g|i] can fire after half the h-matmuls.
                sigF = sp.tile([128, 64], f32, tag="sigF")
                nc.scalar.activation(sigF[:], gB[:, 0:64],
                                     mybir.ActivationFunctionType.Sigmoid)
                sigA = sp.tile([128, 128], f32, tag="sigA")
                nc.scalar.activation(sigA[:], gA[:, 0:128],
                                     mybir.ActivationFunctionType.Sigmoid)
                sigO = sp.tile([128, 64], f32, tag="sigO")
                nc.scalar.activation(sigO[:], gC[:, 0:64],
                                     mybir.ActivationFunctionType.Sigmoid)

                c_new = st.tile([128, 64], f32, tag="c")
                if c_prev is None:
                    nc.vector.scalar_tensor_tensor(
                        c_new[:], sigA[:, 0:64], 0.5, sigA[:, 64:128],
                        mybir.AluOpType.subtract, mybir.AluOpType.mult)
                else:
                    t2 = sp.tile([128, 64], f32, tag="t2")
                    nc.vector.tensor_mul(t2[:], sigF[:], c_prev[:])
                    t1 = sp.tile([128, 64], f32, tag="t1")
                    nc.vector.scalar_tensor_tensor(
                        t1[:], sigA[:, 0:64], 0.5, sigA[:, 64:128],
                        mybir.AluOpType.subtract, mybir.AluOpType.mult)
                    nc.vector.tensor_add(c_new[:], t1[:], t2[:])
                tc_ = sp.tile([128, 64], f32, tag="tc")
                nc.scalar.activation(tc_[:], c_new[:],
                                     mybir.ActivationFunctionType.Tanh,
                                     scale=2.0)
                h = st.tile([128, 64], bf16, tag="h")
                nc.vector.tensor_mul(h[:], sigO[:], tc_[:])

                nc.sync.dma_start(out_d[t], h[:])

                h_prev = h
                c_prev = c_new

    _BUILT[key] = nc
    return nc


def _ensure_split(nc):
    if not getattr(nc, "_waitsplit_done", False):
        _split_sync_waits(nc)
        nc._waitsplit_done = True


def _prep_core_inputs(c, tokens, mask, emb_table, wihT_f, whhT_f, biasL_f,
                      wihT_b, whhT_b, biasL_b, delta, nsteps):
    import ml_dtypes

    bf16 = ml_dtypes.bfloat16
    backward = c >= 4
    s = slice(NB * (c % 4), NB * (c % 4) + NB)
    tok = np.asarray(tokens)[:nsteps, s]
    msk = np.asarray(mask)[:nsteps, s]
    if backward:
        tok = tok[::-1]
        msk = msk[::-1]
    ntiles = nsteps * NB // 128
    toks_c = np.clip(tok, 0, V - 1).astype(np.int32).reshape(ntiles, 128).T
    # pen[j, 192*cc + 16*i + b] = -1e9 * (1 - mask[32*cc + j, b]), i<12
    npc = (nsteps + 31) // 32
    P = np.zeros((npc * 32, NB), np.float32)
    P[:nsteps] = -1e9 * (1.0 - msk.astype(np.float32))
    pen = np.tile(P.reshape(npc, 32, 1, NB), (1, 1, 12, 1))
    pen = pen.transpose(1, 0, 2, 3).reshape(32, npc * 192)
    # sel[j, 128*jj + p] = (j == jj)
    sel = np.repeat(np.eye(32, dtype=np.float32), 128, axis=1)
    return {
        "emb": np.ascontiguousarray(emb_table),
        "toks": np.ascontiguousarray(toks_c),
        "wihT": wihT_b if backward else wihT_f,
        "whhT": whhT_b if backward else whhT_f,
        "biasL": biasL_b if backward else biasL_f,
        "delta": delta,
        "sel": np.ascontiguousarray(sel.astype(bf16)),
        "pen": np.ascontiguousarray(pen.astype(bf16)),
    }


def kernel(tokens, mask, emb_table, W_ih_f, W_hh_f, b_ih_f, b_hh_f,
           W_ih_b, W_hh_b, b_ih_b, b_hh_b, _nsteps=L, _trace=False):
    import ml_dtypes
    from concourse.bass_utils import run_bass_kernel_spmd

    bf16 = ml_dtypes.bfloat16
    tokens = np.asarray(tokens)
    mask = np.asarray(mask, dtype=np.float32)
    emb_table = np.asarray(emb_table, dtype=np.float32)

    perm = _gate_perm()

    # g-gate rows (first 512 after perm) pre-scaled x2: tanh(g)=2*sig(2g)-1
    gscale = np.ones((2048, 1), np.float32)
    gscale[0:512] = 2.0

    def wprep(W):
        Wp = np.asarray(W, np.float32)[perm] * gscale
        return np.ascontiguousarray(Wp.T.reshape(4, 128, 2048).astype(bf16))

    def bprep(bi, bh):
        b = (np.asarray(bi, np.float32) + np.asarray(bh, np.float32))[perm]
        b = b * gscale[:, 0]
        return np.ascontiguousarray(b.reshape(16, 128).astype(bf16))

    wihT_f, whhT_f = wprep(W_ih_f), wprep(W_hh_f)
    wihT_b, whhT_b = wprep(W_ih_b), wprep(W_hh_b)
    biasL_f = bprep(b_ih_f, b_hh_f)
    biasL_b = bprep(b_ih_b, b_hh_b)
    delta = np.ascontiguousarray(
        np.kron(np.eye(16, dtype=np.float32),
                np.ones((1, 16), np.float32)).astype(bf16))

    nsteps = _nsteps
    nc = _build(nsteps)
    _ensure_split(nc)
    in_maps = [
        _prep_core_inputs(c, tokens, mask, emb_table, wihT_f, whhT_f, biasL_f,
                          wihT_b, whhT_b, biasL_b, delta, nsteps)
        for c in range(NCORES)
    ]
    res = run_bass_kernel_spmd(nc, in_maps, core_ids=list(range(NCORES)),
                               trace=_trace)
    out = np.empty((nsteps, B, 2 * H), np.float32)
    for c in range(NCORES):
        o = np.asarray(res.results[c]["out"]).astype(np.float32)
        # o[t, p, 16j+b] -> h[t, b, 128j+p]
        o = o.reshape(nsteps, 128, 4, NB).transpose(0, 3, 2, 1)
        o = np.ascontiguousarray(o).reshape(nsteps, NB, 512)
        s = slice(NB * (c % 4), NB * (c % 4) + NB)
        if c >= 4:
            out[:, s, 512:1024] = o[::-1]
        else:
            out[:, s, 0:512] = o
    kernel._last_results = res
    return out


# revision 5
# speedup vs baseline: 1.0400x; 1.0135x over previous
"""Bidirectional LSTM (L=512, B=64, E=512, H=512 per dir) on 8 NeuronCores.

Strategy (SPMD, zero cross-core communication):
  - Direction x batch sharding: cores 0-3 run the FORWARD direction for
    batches [16c, 16c+16); cores 4-7 run the BACKWARD direction (inputs
    time-reversed on host) for batches [16(c-4), 16(c-4)+16). One Bass
    program for all cores; only the host-prepared inputs differ.
  - Orientation: gate dims live in the PARTITION dim (16 m-tiles of 128,
    gate order [g|i|f|o], each gate 4 h-chunks), batch=16 in the free dim.
    Per step, gates accumulate in three PSUM banks (A=[g|i], B=f, C=o, so
    each sigmoid is only bank-serialized against its own gates' writes):
      inject bias (K=16 matmul vs a static delta pattern) and the padding
      penalty -1e9*(1-mask) (K=32 row-select matmul, i/f/o tiles only),
      then 64 x-part matmuls (Wih^T chunks @ X^T cols) and 64 h-part
      matmuls (f tiles first -- sig_f gates the longest downstream path),
      all N=16. sigmoid(-1e9)=0 reproduces the reference's h,c masking.
  - g rows are pre-scaled x2 on the host so all four gates need only
    sigmoid (tanh(g) = 2*sig(2g)-1), tracking c' = c/2:
      c' = sig_f*c' + (sig_2g-0.5)*sig_i;  h = sig_o*tanh(2c').
  - h is produced as [128 h-dims, 16 batch] bf16 -- exactly the moving
    operand layout the next step's h-matmuls need: no transposes in the
    recurrence.
  - Embedding rows are gathered by indirect DMA (8 steps worth per tile),
    PE-transposed into a persistent bf16 X^T[128, 4, L*16] in SBUF,
    pipelined ahead of the recurrence.
  - All matmul operands bf16; the h-path (sig_o, tanh(2c')) is bf16 for
    the DVE 2x mode; PSUM accumulation, the c state and its inputs stay
    fp32 (bf16 there breaks the 512-step error budget).
"""

import os
import sys

sys.path.insert(0, "/opt/trn_rl_repo")

import numpy as np

L, B, E, V = 512, 64, 512, 32000
H = 512            # hidden per direction
NB = 16            # batch per core
NCORES = 8
NM = 16            # m-tiles (2048 gate dims / 128)

_BUILT = {}


def _split_sync_waits(nc, max_waits=1):
    """This container's walrus rejects >1 sync-wait per instruction
    (CoreV3GenImpl setupSyncWait). Split extras onto preceding same-engine
    NoOps. Keep the *most recently required* wait (highest wait_value
    relative to that semaphore's final count) on the instruction itself, so
    the NoOps carry long-satisfied waits and drain through the sequencer
    without stalling the critical path."""
    import concourse.mybir as mybir

    # final tick per semaphore id + whether it is only updated by DMA-ish
    # instructions (those sems are long-satisfied WAR guards, never the
    # chain-critical producer)
    total = {}
    dma_only = {}
    for fn in nc.m.functions:
        for blk in fn.blocks:
            for inst in blk.instructions:
                si = inst.sync_info
                if si is None:
                    continue
                is_dma = "DMA" in type(inst).__name__ or "Dma" in type(
                    inst).__name__
                for u in si.on_update:
                    v = total.get(u.id, 0)
                    total[u.id] = v + (u.update_value or 1)
                    dma_only[u.id] = dma_only.get(u.id, True) and is_dma

    def recency(w):
        t = total.get(w.id, 0)
        r = (w.wait_value or 0) / t if t else 0.0
        return (0 if dma_only.get(w.id, False) else 1, r)

    ctr = 0
    for fn in nc.m.functions:
        for blk in fn.blocks:
            out = []
            changed = False
            for inst in blk.instructions:
                si = inst.sync_info
                if si is not None and si.on_wait and len(si.on_wait) > max_waits:
                    waits = sorted(si.on_wait, key=recency)
                    extra, keep = waits[:-max_waits], waits[-max_waits:]
                    for i in range(0, len(extra), max_waits):
                        ctr += 1
                        nop = mybir.InstNoOp(
                            name=f"bass_waitsplit_{ctr}", ins=[], outs=[])
                        nop.engine = inst.engine
                        nop.sync_info = mybir.SyncInfo(
                            on_wait=extra[i:i + max_waits], on_update=[])
                        out.append(nop)
                    si.on_wait = keep
                    changed = True
                out.append(inst)
            if changed:
                blk.instructions[:] = out


# gate order [g, i, f, o] referencing reference row blocks i=0:512,
# f=512:1024, g=1024:1536, o=1536:2048. g,i first so sigmoid over [g|i]
# can start after only half the h-part matmuls.
_GATE_BASES = (1024, 0, 512, 1536)


def _gate_perm():
    return np.concatenate([np.arange(b, b + 512) for b in _GATE_BASES])


def _build(nsteps=L):
    key = (nsteps, NB)
    if key in _BUILT:
        return _BUILT[key]
    import concourse.bass as bass
    import concourse.mybir as mybir
    import concourse.tile as tile
    from concourse.masks import make_identity

    f32 = mybir.dt.float32
    bf16 = mybir.dt.bfloat16
    ntiles = nsteps * NB // 128   # gather row-tiles (8 steps each)
    npc = (nsteps + 31) // 32     # penalty col-chunks (32 steps each)

    nc = bass.Bass()
    emb = nc.dram_tensor("emb", [V, E], f32, kind="ExternalInput")
    toks = nc.dram_tensor("toks", [128, ntiles], mybir.dt.int32,
                          kind="ExternalInput")
    wihT_d = nc.dram_tensor("wihT", [4, 128, 2048], bf16, kind="ExternalInput")
    whhT_d = nc.dram_tensor("whhT", [4, 128, 2048], bf16, kind="ExternalInput")
    biasL_d = nc.dram_tensor("biasL", [16, 128], bf16, kind="ExternalInput")
    delta_d = nc.dram_tensor("delta", [16, 256], bf16, kind="ExternalInput")
    sel_d = nc.dram_tensor("sel", [32, 32 * 128], bf16, kind="ExternalInput")
    pen_d = nc.dram_tensor("pen", [32, npc * 192], bf16, kind="ExternalInput")
    out_d = nc.dram_tensor("out", [nsteps, 128, 64], bf16,
                           kind="ExternalOutput")

    PF = 2  # prefetch distance in row-tiles (gather 2*PF ahead, transpose PF)

    with tile.TileContext(nc) as tc:
        with (
            tc.tile_pool(name="persist", bufs=1) as pp,
            tc.tile_pool(name="gat", bufs=2 * PF + 1) as gat,
            tc.tile_pool(name="step", bufs=3) as sp,
            tc.tile_pool(name="state", bufs=3) as st,
            tc.tile_pool(name="psA", bufs=2, space="PSUM") as psA,
            tc.tile_pool(name="psB", bufs=2, space="PSUM") as psB,
            tc.tile_pool(name="psC", bufs=2, space="PSUM") as psC,
            tc.tile_pool(name="pstr", bufs=2, space="PSUM") as pstr,
        ):
            # ---- persistent SBUF ----
            wihT = pp.tile([128, 4, 2048], bf16)
            whhT = pp.tile([128, 4, 2048], bf16)
            for k in range(4):
                nc.sync.dma_start(wihT[:, k, :], wihT_d[k])
                nc.sync.dma_start(whhT[:, k, :], whhT_d[k])
            biasL = pp.tile([16, 128], bf16)
            nc.sync.dma_start(biasL[:], biasL_d[:])
            delta = pp.tile([16, 256], bf16)
            nc.sync.dma_start(delta[:], delta_d[:])
            pen = pp.tile([32, npc * 192], bf16)
            nc.sync.dma_start(pen[:], pen_d[:])
            sel = pp.tile([32, 32 * 128], bf16)
            nc.sync.dma_start(sel[:], sel_d[:])
            toks_t = pp.tile([128, ntiles], mybir.dt.int32)
            nc.sync.dma_start(toks_t[:], toks[:])
            ident = pp.tile([128, 128], f32)
            make_identity(nc, ident[:])
            xT = pp.tile([128, 4, nsteps * NB], bf16)

            def gather(r):
                xt = gat.tile([128, E], f32, tag="xt")
                nc.gpsimd.indirect_dma_start(
                    out=xt[:], out_offset=None, in_=emb[:],
                    in_offset=bass.IndirectOffsetOnAxis(
                        ap=toks_t[:, r:r + 1], axis=0))
                return xt

            def transpose_tile(r, xt):
                # copies stay off ACT: ACT is the chain-critical engine
                for k in range(4):
                    trp = pstr.tile([128, 128], f32, space="PSUM")
                    nc.tensor.transpose(
                        out=trp[:], in_=xt[:, 128 * k:128 * (k + 1)],
                        identity=ident[:])
                    nc.vector.tensor_copy(xT[:, k, 128 * r:128 * (r + 1)],
                                          trp[:])

            xts = {r: gather(r) for r in range(min(2 * PF, ntiles))}
            for r in range(min(PF, ntiles)):
                transpose_tile(r, xts.pop(r))

            h_prev = None
            c_prev = None
            for t in range(nsteps):
                if t % 8 == 0:
                    r = t // 8
                    if r + 2 * PF < ntiles:
                        xts[r + 2 * PF] = gather(r + 2 * PF)
                    if r + PF < ntiles:
                        transpose_tile(r + PF, xts.pop(r + PF))

                # Gate groups in separate PSUM banks so each sigmoid is only
                # bank-serialized against its own gates' matmul writes:
                # bank A = [g|i] (m 0..7), bank B = f (m 8..11), C = o (12..15)
                gA = psA.tile([128, 512], f32, space="PSUM")
                gB = psB.tile([128, 512], f32, space="PSUM")
                gC = psC.tile([128, 512], f32, space="PSUM")
                banks = [gA] * 8 + [gB] * 4 + [gC] * 4
                boff = [16 * m for m in range(8)] + [16 * m for m in range(4)] * 2

                def gslice(m):
                    return banks[m][:, boff[m]:boff[m] + 16]

                cc = 192 * (t // 32)
                selt = sel[:, 128 * (t % 32):128 * (t % 32) + 128]
                # bias inject: out[p, 16m+b] += biasL[m, p]
                nc.tensor.matmul(gA[:, 0:128], biasL[:, :], delta[:, 0:128],
                                 start=True, stop=False, skip_group_check=True)
                nc.tensor.matmul(gB[:, 0:64], biasL[:, :], delta[:, 128:192],
                                 start=True, stop=False, skip_group_check=True)
                nc.tensor.matmul(gC[:, 0:64], biasL[:, :], delta[:, 192:256],
                                 start=True, stop=False, skip_group_check=True)
                # penalty inject on i/f/o tiles (m=4..15): += P[t, b]
                # lhsT selects pen row t%32 (sel row t%32 is all-ones)
                nc.tensor.matmul(gA[:, 64:128], selt, pen[:, cc:cc + 64],
                                 start=False, stop=False, skip_group_check=True)
                nc.tensor.matmul(gB[:, 0:64], selt, pen[:, cc + 64:cc + 128],
                                 start=False, stop=False, skip_group_check=True)
                nc.tensor.matmul(gC[:, 0:64], selt, pen[:, cc + 128:cc + 192],
                                 start=False, stop=False, skip_group_check=True)
                # x part: gates += Wih^T[k,m] @ xT[k][:, t]
                for k in range(4):
                    rhs = xT[:, k, NB * t:NB * (t + 1)]
                    for m in range(NM):
                        nc.tensor.matmul(
                            gslice(m),
                            wihT[:, k, 128 * m:128 * (m + 1)], rhs,
                            start=False,
                            stop=(h_prev is None and k == 3),
                            skip_group_check=True)
                # h part: gates += Whh^T[k,m] @ h_prev[:, k].
                # f tiles (m 8..11) first: sig_f -> t2 is the longest
                # downstream path; then g,i (t1), o last.
                if h_prev is not None:
                    for m in (8, 9, 10, 11, 0, 1, 2, 3, 4, 5, 6, 7,
                              12, 13, 14, 15):
                        for k in range(4):
                            nc.tensor.matmul(
                                gslice(m),
                                whhT[:, k, 128 * m:128 * (m + 1)],
                                h_prev[:, 16 * k:16 * (k + 1)],
                                start=False, stop=(k == 3),
                                skip_group_check=True)

                # Gate cols: g 0:64, i 64:128, f 128:192, o 192:256.
                # g rows were pre-scaled x2 on the host, so
                # tanh(g) = 2*sig(2g) - 1; we track c' = c/2:
                #   c' = sig_f*c'_prev + (sig_2g-0.5)*sig_i, h = sig_o*tanh(2c')
                # Two sigmoid halves: [g|i] can fire after half the h-matmuls.
                sigF = sp.tile([128, 64], f32, tag="sigF")
                nc.scalar.activation(sigF[:], gB[:, 0:64],
                                     mybir.ActivationFunctionType.Sigmoid)
                sigA = sp.tile([128, 128], f32, tag="sigA")
                nc.scalar.activation(sigA[:], gA[:, 0:128],
                                     mybir.ActivationFunctionType.Sigmoid)
                sigO = sp.tile([128, 64], bf16, tag="sigO")
                nc.scalar.activation(sigO[:], gC[:, 0:64],
                                     mybir.ActivationFunctionType.Sigmoid)

                c_new = st.tile([128, 64], f32, tag="c")
                if c_prev is None:
                    nc.vector.scalar_tensor_tensor(
                        c_new[:], sigA[:, 0:64], 0.5, sigA[:, 64:128],
                        mybir.AluOpType.subtract, mybir.AluOpType.mult)
                else:
                    t2 = sp.tile([128, 64], f32, tag="t2")
                    nc.vector.tensor_mul(t2[:], sigF[:], c_prev[:])
                    t1 = sp.tile([128, 64], f32, tag="t1")
                    nc.vector.scalar_tensor_tensor(
                        t1[:], sigA[:, 0:64], 0.5, sigA[:, 64:128],
                        mybir.AluOpType.subtract, mybir.AluOpType.mult)
                    nc.vector.tensor_add(c_new[:], t1[:], t2[:])
                tc_ = sp.tile([128, 64], bf16, tag="tc")
                nc.scalar.activation(tc_[:], c_new[:],
                                     mybir.ActivationFunctionType.Tanh,
                                     scale=2.0)
                h = st.tile([128, 64], bf16, tag="h")
                nc.vector.tensor_mul(h[:], sigO[:], tc_[:])

                nc.sync.dma_start(out_d[t], h[:])

                h_prev = h
                c_prev = c_new

    _BUILT[key] = nc
    return nc


def _ensure_split(nc):
    if not getattr(nc, "_waitsplit_done", False):
        _split_sync_waits(nc)
        nc._waitsplit_done = True


def _prep_core_inputs(c, tokens, mask, emb_table, wihT_f, whhT_f, biasL_f,
                      wihT_b, whhT_b, biasL_b, delta, nsteps):
    import ml_dtypes

    bf16 = ml_dtypes.bfloat16
    backward = c >= 4
    s = slice(NB * (c % 4), NB * (c % 4) + NB)
    tok = np.asarray(tokens)[:nsteps, s]
    msk = np.asarray(mask)[:nsteps, s]
    if backward:
        tok = tok[::-1]
        msk = msk[::-1]
    ntiles = nsteps * NB // 128
    toks_c = np.clip(tok, 0, V - 1).astype(np.int32).reshape(ntiles, 128).T
    # pen[j, 192*cc + 16*i + b] = -1e9 * (1 - mask[32*cc + j, b]), i<12
    npc = (nsteps + 31) // 32
    P = np.zeros((npc * 32, NB), np.float32)
    P[:nsteps] = -1e9 * (1.0 - msk.astype(np.float32))
    pen = np.tile(P.reshape(npc, 32, 1, NB), (1, 1, 12, 1))
    pen = pen.transpose(1, 0, 2, 3).reshape(32, npc * 192)
    # sel[j, 128*jj + p] = (j == jj)
    sel = np.repeat(np.eye(32, dtype=np.float32), 128, axis=1)
    return {
        "emb": np.ascontiguousarray(emb_table),
        "toks": np.ascontiguousarray(toks_c),
        "wihT": wihT_b if backward else wihT_f,
        "whhT": whhT_b if backward else whhT_f,
        "biasL": biasL_b if backward else biasL_f,
        "delta": delta,
        "sel": np.ascontiguousarray(sel.astype(bf16)),
        "pen": np.ascontiguousarray(pen.astype(bf16)),
    }


def kernel(tokens, mask, emb_table, W_ih_f, W_hh_f, b_ih_f, b_hh_f,
           W_ih_b, W_hh_b, b_ih_b, b_hh_b, _nsteps=L, _trace=False):
    import ml_dtypes
    from concourse.bass_utils import run_bass_kernel_spmd

    bf16 = ml_dtypes.bfloat16
    tokens = np.asarray(tokens)
    mask = np.asarray(mask, dtype=np.float32)
    emb_table = np.asarray(emb_table, dtype=np.float32)

    perm = _gate_perm()

    # g-gate rows (first 512 after perm) pre-scaled x2: tanh(g)=2*sig(2g)-1
    gscale = np.ones((2048, 1), np.float32)
    gscale[0:512] = 2.0

    def wprep(W):
        Wp = np.asarray(W, np.float32)[perm] * gscale
        return np.ascontiguousarray(Wp.T.reshape(4, 128, 2048).astype(bf16))

    def bprep(bi, bh):
        b = (np.asarray(bi, np.float32) + np.asarray(bh, np.float32))[perm]
        b = b * gscale[:, 0]
        return np.ascontiguousarray(b.reshape(16, 128).astype(bf16))

    wihT_f, whhT_f = wprep(W_ih_f), wprep(W_hh_f)
    wihT_b, whhT_b = wprep(W_ih_b), wprep(W_hh_b)
    biasL_f = bprep(b_ih_f, b_hh_f)
    biasL_b = bprep(b_ih_b, b_hh_b)
    delta = np.ascontiguousarray(
        np.kron(np.eye(16, dtype=np.float32),
                np.ones((1, 16), np.float32)).astype(bf16))

    nsteps = _nsteps
    nc = _build(nsteps)
    _ensure_split(nc)
    in_maps = [
        _prep_core_inputs(c, tokens, mask, emb_table, wihT_f, whhT_f, biasL_f,
                          wihT_b, whhT_b, biasL_b, delta, nsteps)
        for c in range(NCORES)
    ]
    res = run_bass_kernel_spmd(nc, in_maps, core_ids=list(range(NCORES)),
                               trace=_trace)
    out = np.empty((nsteps, B, 2 * H), np.float32)
    for c in range(NCORES):
        o = np.asarray(res.results[c]["out"]).astype(np.float32)
        # o[t, p, 16j+b] -> h[t, b, 128j+p]
        o = o.reshape(nsteps, 128, 4, NB).transpose(0, 3, 2, 1)
        o = np.ascontiguousarray(o).reshape(nsteps, NB, 512)
        s = slice(NB * (c % 4), NB * (c % 4) + NB)
        if c >= 4:
            out[:, s, 512:1024] = o[::-1]
        else:
            out[:, s, 0:512] = o
    kernel._last_results = res
    return out
